# revision 2
# baseline (speedup 1.0000x reference)
"""Trainium2 Bass kernel for DigitConvolutionalModel.

Computes, for x [32768, 784] viewed as 28x28 images:
    feat = relu(conv3x3_valid(x))      # [B, 676]
    out  = feat @ W + b                # [B, 10]

Strategy (pure data parallel over 8 cores, 4096 rows each):
  - Host pre-casts x to bf16 (same numerics as the previous in-DMA
    fp32->bf16 cast, half the HBM traffic) and pre-transposes each
    core's shard to pixel-major superchunks xh [4*784, 1024] so the
    contraction dims sit on SBUF partitions and every DRAM row is a
    2 KB contiguous DMA descriptor.
  - The 3x3 conv is a banded matmul y^T = C^T @ x^T using two constant
    blocks C1/C2 [112, 128] built on host from conv_w: input rows are
    tiled 4 image rows (112 pixels) per partition group, output rows
    4 conv rows (104 pixels, padded to 128) per PSUM tile.
  - ReLU evacuates PSUM -> SBUF bf16 (split between ScalarE and VectorE).
  - The 676->10 linear layer is column-tiled on the PE array: the 7
    feature-tile matmuls are spread over 3 column strips (partitions
    0-31 / 32-63 / 64-95 of one PSUM tile) that run concurrently, each
    accumulating a partial sum; a final matmul against a 0/1 selection
    matrix reduces the strips to out^T [10, chunk], and bias is added
    during the PSUM->SBUF copy.
  - Device emits out^T [10, 4096]; host transposes back.

Walrus accepts only ONE semaphore wait per engine instruction, so the
kernel is arranged so every instruction needs at most one: constants are
pre-touched by tiny warm-up ops, each x sub-block DMA is absorbed by a
touch matmul placed just before its first consumer, engine assignments
keep wait-pairs on the same semaphore so they merge, redundant
same-engine waits Tile emits are stripped, and the kernel-tail drain is
split into single-wait drains.
"""

import numpy as np

try:
    from concourse import bass, mybir
    from concourse.tile import TileContext
    from concourse.bass_utils import run_bass_kernel_spmd
except ImportError:  # path used when concourse is not already importable
    import sys

    sys.path.insert(0, "/opt/trn_rl_repo")
    from concourse import bass, mybir
    from concourse.tile import TileContext
    from concourse.bass_utils import run_bass_kernel_spmd

from concourse.vector_clock import ScopedClock


def _patched_drain_and_barrier(self, tick_clock, wait_clock):
    """Replacement for TileContext._drain_and_barrier: walrus rejects
    instructions carrying more than one sync wait, but the kernel-tail
    drain aggregates a wait per logical proc. Emit a chain of
    single-wait drains on the sync queue instead."""
    nc = self.nc
    drain_inst = nc.sync.drain()
    wait_clock.add_sem_waits(
        drain_inst.ins, ScopedClock({None: tick_clock.global_clock})
    )
    si = drain_inst.ins.sync_info
    waits = list(si.on_wait or []) if si else []
    if len(waits) > 1:
        drain_inst.ins.sync_info = mybir.SyncInfo(
            on_wait=waits[:1], on_update=si.on_update
        )
        for w in waits[1:]:
            extra = nc.sync.drain()
            esi = extra.ins.sync_info
            extra.ins.sync_info = mybir.SyncInfo(
                on_wait=[w], on_update=(esi.on_update if esi else [])
            )
    nc.all_engine_barrier()
    popped = nc._tile_sem_poison_stack.pop()
    assert popped is self._sem_poison
    nc.clear_and_free_semaphores(list(self.sems.allocated().values()))
    nc.all_engine_barrier()


TileContext._drain_and_barrier = _patched_drain_and_barrier

N_CORES = 8
B = 32768
B_CORE = B // N_CORES  # 4096
NSC = 4  # superchunks of 1024 batch columns (2 KB bf16 DMA rows)
SC = 1024
NT = 7  # pixel-group tiles of 4 image rows (112 pixels); 7*4 = 28 rows
NL = 3  # linear column strips (PE col groups at partitions 0/32/64)
N_JUNK = 12  # PE HAM warm-up matmuls (N=256) before real work

F32 = mybir.dt.float32
BF16 = mybir.dt.bfloat16
RELU = mybir.ActivationFunctionType.Relu
IDENT = mybir.ActivationFunctionType.Identity

# cpk column layout: c1 | c2 | wp_pad[t]*7 | S
CPK_C2 = 128
CPK_WP = 256
CPK_S = 256 + 32 * NT  # 480
CPK_COLS = CPK_S + 10  # 490

_NC_CACHE = {}

# linear strip membership: tile t -> strip t % NL
_STRIP = [t % NL for t in range(NT)]
_STRIP_FIRST = [min(t for t in range(NT) if _STRIP[t] == j) for j in range(NL)]
_STRIP_LAST = [max(t for t in range(NT) if _STRIP[t] == j) for j in range(NL)]


def _build_nc():
    nc = bass.Bass(
        "TRN2", target_bir_lowering=False, debug=False, num_devices=1
    )

    # superchunk-major pixel-major input: superchunk s occupies rows
    # 784s..784s+783 (row within superchunk = pixel), cols = batch within
    # superchunk — each superchunk is one dense 1.57 MB block so the HBM
    # read stream stays sequential, in 2 KB rows.
    xh = nc.dram_tensor("xh", [NSC * 784, SC], BF16, kind="ExternalInput")
    cpk_d = nc.dram_tensor("cpk", [128, CPK_COLS], BF16, kind="ExternalInput")
    bias_d = nc.dram_tensor("bias_in", [10, 1], F32, kind="ExternalInput")
    out_t = nc.dram_tensor("out_t", [10, B_CORE], F32, kind="ExternalOutput")

    with TileContext(nc) as tc:
        with (
            tc.tile_pool(name="const", bufs=1) as cpool,
            tc.tile_pool(name="xc", bufs=1) as xpool,
            tc.tile_pool(name="ry_a", bufs=4) as rypool_a,
            tc.tile_pool(name="ry_v", bufs=5) as rypool_v,
            tc.tile_pool(name="psb", bufs=2) as pspool_sb,
            tc.tile_pool(name="outT", bufs=1) as opool,
            tc.tile_pool(name="yps_a", bufs=2, space="PSUM") as ypool_a,
            tc.tile_pool(name="yps_v", bufs=2, space="PSUM") as ypool_v,
            tc.tile_pool(name="pps", bufs=1, space="PSUM") as ppool,
            tc.tile_pool(name="ops", bufs=2, space="PSUM") as opsum,
            tc.tile_pool(name="warmp", bufs=1, space="PSUM") as warmpool,
        ):
            # sub-block pixel-group boundaries per superchunk: early
            # superchunks load in pieces so the conv pipeline starts as
            # soon as the first blocks land.
            splits = {0: (0, 2, 4, NT), 1: (0, 3, NT)}

            def load_block(tile, s, lo, hi):
                blk = bass.AP(
                    xh,
                    (784 * s + 112 * lo) * SC,
                    [[SC, 112], [112 * SC, hi - lo], [1, SC]],
                )
                nc.gpsimd.dma_start(tile[:, SC * lo : SC * hi], blk)

            # The tiny bias DMA goes first to absorb the SWDGE pipeline's
            # cold start; then superchunk 0's first sub-load, the packed
            # constants, and the remaining blocks.
            bias_sb = cpool.tile([10, 1], F32, tag="bias")
            nc.gpsimd.dma_start(bias_sb[:], bias_d.ap())
            xc = [
                xpool.tile([112, NT * SC], BF16, tag=f"xc{s}", name=f"xc{s}")
                for s in range(NSC)
            ]
            load_block(xc[0], 0, 0, 2)
            cpk_sb = cpool.tile([128, CPK_COLS], BF16, tag="cpk")
            nc.gpsimd.dma_start(cpk_sb[:], cpk_d.ap())
            c1_sb = cpk_sb[0:112, 0:128]
            c2_sb = cpk_sb[0:112, CPK_C2 : CPK_C2 + 128]
            s_sb = cpk_sb[0 : 32 * NL, CPK_S : CPK_S + 10]
            for s in range(NSC):
                sp = splits.get(s, (0, NT))
                for lo, hi in zip(sp, sp[1:]):
                    if s == 0 and lo == 0:
                        continue  # already issued above
                    load_block(xc[s], s, lo, hi)

            outT_sb = opool.tile([10, B_CORE], F32, tag="outT")

            # PE HAM warm-up: the PE clock-gate only lifts to 2.4 GHz after
            # ~3.4us of sustained activity. Fill the initial DMA-wait window
            # with junk matmuls so the real matmuls run warm. The memset
            # runs on the otherwise-idle VectorE.
            junk = cpool.tile([112, 256], BF16, tag="junk")
            nc.vector.memset(junk[:], 0.0)
            warm = warmpool.tile([8, 256], F32, tag="warm")
            with tc.high_priority():
                for _ in range(N_JUNK):
                    nc.tensor.matmul(warm[:], junk[:, 0:8], junk[:])

            # Pre-touch the constants with a tiny op so real instructions'
            # dependency on their DMA is satisfied by engine program order.
            nc.tensor.matmul(warm[0:4, 0:4], c1_sb[:, 0:4], c1_sb[:, 0:4])
            warm_act = cpool.tile([10, 1], F32, tag="warm_act")
            nc.scalar.activation(warm_act[:], bias_sb[:], IDENT, bias=bias_sb[:])

            def touch(s, lo):
                # Absorbs the sub-block's DMA wait on PE so the conv
                # matmuls only carry their PSUM-slot wait.
                nc.tensor.matmul(
                    warm[0:4, 0:4],
                    xc[s][:, SC * lo : SC * lo + 4],
                    xc[s][:, SC * lo : SC * lo + 4],
                )

            for s in range(NSC):
                sp = splits.get(s, (0, NT))
                # block k covers pixel groups sp[k]..sp[k+1]-1; conv tile t
                # reads groups t and t+1, so block k must be touched before
                # the first tile t with t+1 >= sp[k] (i.e. t >= sp[k]-1).
                touch_before = {max(0, sp[k] - 1): k for k in range(len(sp) - 1)}
                for h in range(2):
                    col = lambda t: SC * t + 512 * h
                    rys = []
                    for t in range(NT):
                        if h == 0 and t in touch_before:
                            touch(s, sp[touch_before[t]])
                        on_act = t in (0, 2, 4)
                        yps = (ypool_a if on_act else ypool_v).tile(
                            [128, 512], F32, tag="yps"
                        )
                        nc.tensor.matmul(
                            yps[:],
                            c1_sb,
                            xc[s][:, col(t) : col(t) + 512],
                            start=True,
                            stop=(t == NT - 1),
                        )
                        if t < NT - 1:
                            nc.tensor.matmul(
                                yps[:],
                                c2_sb,
                                xc[s][:, col(t + 1) : col(t + 1) + 512],
                                start=False,
                                stop=True,
                            )
                        ry = (rypool_a if on_act else rypool_v).tile(
                            [128, 512], BF16, tag="ry"
                        )
                        if on_act:
                            nc.scalar.activation(ry[:], yps[:], RELU)
                        else:
                            nc.vector.tensor_relu(ry[:], yps[:])
                        rys.append(ry)

                    # linear partials: strip j accumulates tiles t≡j (mod NL)
                    # concurrently on PE col groups 0/32/64.
                    pps = ppool.tile([32 * NL, 512], F32, tag="pps")
                    for t in range(NT):
                        j = _STRIP[t]
                        nc.tensor.matmul(
                            pps[32 * j : 32 * (j + 1), :],
                            cpk_sb[:, CPK_WP + 32 * t : CPK_WP + 32 * (t + 1)],
                            rys[t][:],
                            start=(t == _STRIP_FIRST[j]),
                            stop=(t == _STRIP_LAST[j]),
                        )
                    psb = pspool_sb.tile([32 * NL, 512], BF16, tag="psb")
                    nc.scalar.activation(psb[:], pps[:], IDENT)
                    ops = opsum.tile([10, 512], F32, tag="ops")
                    nc.tensor.matmul(ops[:], s_sb, psb[:], start=True, stop=True)

                    off = SC * s + 512 * h
                    nc.scalar.activation(
                        outT_sb[:, off : off + 512],
                        ops[:],
                        IDENT,
                        bias=bias_sb[:],
                    )
                    # Output DMAs on the otherwise-idle SP queue: writing as
                    # compute finishes hides the HBM write-receipt latency of
                    # all but the last chunk.
                    nc.sync.dma_start(
                        out_t.ap()[:, off : off + 512],
                        outT_sb[:, off : off + 512],
                    )

    _strip_self_waits(nc)
    return nc


_ENGINE_SEM_PREFIX = {
    mybir.EngineType.PE: "PE_",
    mybir.EngineType.Activation: "Activation_",
    mybir.EngineType.DVE: "DVE_",
    mybir.EngineType.Pool: "Pool_",
    mybir.EngineType.SP: "SP_",
}


def _strip_self_waits(nc):
    """Drop semaphore waits an instruction holds on its OWN engine's
    completion counter. Engines execute their queue strictly in order, so
    a wait on the own-engine sem at a value covered by program order is
    redundant — but Tile still emits it, and walrus rejects compute
    instructions carrying more than one sync wait."""
    for fn in nc.m.functions:
        for blk in fn.blocks:
            for inst in blk.instructions:
                tn = type(inst).__name__
                if tn in ("InstDrain", "InstEventSemaphore", "InstDMACopy"):
                    continue
                si = inst.sync_info
                if si is None or not si.on_wait or len(si.on_wait) < 2:
                    continue
                pref = _ENGINE_SEM_PREFIX.get(inst.engine)
                if pref is None:
                    continue
                kept = [w for w in si.on_wait if not w.ant_name.startswith(pref)]
                if len(kept) != len(si.on_wait):
                    inst.sync_info = mybir.SyncInfo(
                        on_wait=kept, on_update=si.on_update
                    )


def _build_consts(conv_w, W, b):
    conv_w = np.asarray(conv_w, np.float32)
    W = np.asarray(W, np.float32)
    b = np.asarray(b, np.float32)

    # C1: input rows 4t+rl (rl 0..3) -> output conv rows 4t+il (il 0..3)
    # C2: input rows 4(t+1)+rl      -> output conv rows 4t+il
    c1 = np.zeros((112, 128), np.float32)
    c2 = np.zeros((112, 128), np.float32)
    for rl in range(4):
        for c in range(28):
            for il in range(4):
                for j in range(26):
                    dj = c - j
                    if not (0 <= dj <= 2):
                        continue
                    di = rl - il
                    if 0 <= di <= 2:
                        c1[rl * 28 + c, il * 26 + j] = conv_w[di, dj]
                    di2 = 4 + rl - il
                    if 0 <= di2 <= 2:
                        c2[rl * 28 + c, il * 26 + j] = conv_w[di2, dj]

    # W packed: block t holds rows for conv-output rows 4t..4t+3, padded
    # to 32 stationary columns so each linear matmul covers its whole
    # 32-partition strip (zero-filling the unused partial-sum rows).
    wp = np.zeros((128, 32 * NT), np.float32)
    for t in range(6):
        wp[0:104, 32 * t : 32 * t + 10] = W[104 * t : 104 * (t + 1)]
    wp[0:52, 32 * 6 : 32 * 6 + 10] = W[624:676]

    # strip-reduce selection: partial strip j rows 32j..32j+9 -> out rows
    sel = np.zeros((128, 10), np.float32)
    for j in range(NL):
        for i in range(10):
            sel[32 * j + i, i] = 1.0

    import ml_dtypes

    cpk = np.zeros((128, CPK_COLS), np.float32)
    cpk[0:112, 0:128] = c1
    cpk[0:112, CPK_C2 : CPK_C2 + 128] = c2
    cpk[:, CPK_WP : CPK_WP + 32 * NT] = wp
    cpk[:, CPK_S : CPK_S + 10] = sel
    return cpk.astype(ml_dtypes.bfloat16), b.reshape(10, 1).copy()


def _run(inputs, trace=False):
    import ml_dtypes

    x = np.asarray(inputs["x"], np.float32)
    conv_w = inputs["conv_w"]
    W = inputs["W"]
    b = inputs["b"]

    if "nc" not in _NC_CACHE:
        _NC_CACHE["nc"] = _build_nc()
    nc = _NC_CACHE["nc"]

    cpk, bias = _build_consts(conv_w, W, b)

    xbf = x.astype(ml_dtypes.bfloat16)
    in_maps = []
    for c in range(N_CORES):
        shard = xbf[c * B_CORE : (c + 1) * B_CORE]  # [4096, 784] bf16
        # [4, 1024, 784] -> [4, 784, 1024]: superchunk-major, pixel rows
        xhc = np.ascontiguousarray(
            shard.reshape(NSC, SC, 784).transpose(0, 2, 1)
        ).reshape(NSC * 784, SC)
        in_maps.append({"xh": xhc, "cpk": cpk, "bias_in": bias})

    res = run_bass_kernel_spmd(
        nc, in_maps, core_ids=list(range(N_CORES)), trace=trace
    )
    out = np.concatenate(
        [np.asarray(res.results[c]["out_t"]).T for c in range(N_CORES)], axis=0
    )
    return out, res


def kernel(**inputs) -> np.ndarray:
    return _run(inputs, trace=False)[0]
